# revision 1
# baseline (speedup 1.0000x reference)
"""HarmonicNoiseSynth Trainium2 kernel.

Sharding: 8 cores = 4 batches x 2 harmonic halves (64 harmonics each).
Each core also handles 16 of the 32 noise bands; cores with j==0 compute the
modulator (noise-burst) path for harmonics 0..3. Host combines partials.

Per-core layout: harmonic rows split into 4 h-groups of 16; partitions hold
(h_local, tb) = h_local*8 + tb where tb indexes 8 time slices of 8192; free
dim is time within the slice, processed in 8 chunks of 1024.

Phase accumulation runs in Hz units (scan of masked frequencies) with
mod-48000 reductions at chunk boundaries, slice carries via a PE shift-matrix
matmul, and a final round-reduce; cos(x) = Sin(k*m + k*carry_term) with
k = 2*pi/48000 folded into the activation scale so the Sin argument stays in
[-pi, pi] where the LUT is valid. Per-time sums over harmonics/bands/
modulators are PE matmuls with block-diagonal indicator matrices (contraction
over partitions).
"""
import sys

sys.path.insert(0, "/opt/trn_rl_repo")

import numpy as np

import concourse.bass as bass
import concourse.mybir as mybir
from concourse.tile import TileContext
from concourse.bass_utils import run_bass_kernel_spmd

F = mybir.dt.float32
SR = 48000.0
K = float(2.0 * np.pi / SR)
INV_SR = float(1.0 / SR)
RC = float(1.5 * 2**23)  # fp32 round-to-nearest-integer magic constant
B, H, NB, T = 4, 128, 32, 65536
NTB = 8          # time slices on partitions
TS = T // NTB    # 8192 per slice
TC = 1024        # chunk columns
NCH = TS // TC   # 8 chunks
NG = 4           # h-groups of 16 harmonics
HG = 16
NM = 4           # modulators

_CACHE = {}


def _round_cols(nc, pool, out_col, in_col, modulus):
    """out = in - modulus*round(in/modulus) on a [128,1] column (3 tiny DVE ops)."""
    t1 = pool.tile([128, 1], F, tag="rc1")
    nc.vector.tensor_scalar(out=t1, in0=in_col, scalar1=float(1.0 / modulus),
                            scalar2=RC, op0=mybir.AluOpType.mult,
                            op1=mybir.AluOpType.add)
    t2 = pool.tile([128, 1], F, tag="rc2")
    nc.vector.tensor_scalar(out=t2, in0=t1, scalar1=RC, scalar2=float(-modulus),
                            op0=mybir.AluOpType.subtract,
                            op1=mybir.AluOpType.mult)
    # out = in + (-modulus * round) ; t2 = -modulus*round
    nc.vector.tensor_add(out=out_col, in0=in_col, in1=t2)


def _split_multiwaits(nc):
    """This walrus build supports ONE sync wait per instruction; hoist extras
    onto single-wait NoOps inserted before the offending instruction."""
    ctr = 0
    for f in nc.m.functions:
        for bb in f.blocks:
            insts = list(bb.instructions)
            if not any(i.sync_info is not None and len(i.sync_info.on_wait) > 1
                       for i in insts):
                continue
            new = []
            for inst in insts:
                si = inst.sync_info
                if si is not None and len(si.on_wait) > 1:
                    waits = list(si.on_wait)
                    for w in waits[:-1]:
                        ctr += 1
                        nop = mybir.InstNoOp(name=f"mwsplit_{ctr}",
                                             engine=inst.engine)
                        nop.sync_info = mybir.SyncInfo(on_wait=[w], on_update=[])
                        new.append(nop)
                    inst.sync_info = mybir.SyncInfo(on_wait=[waits[-1]],
                                                    on_update=list(si.on_update))
                new.append(inst)
            bb.instructions = new
    return ctr


def _build():
    nc = bass.Bass("TRN2")
    HN = H // 2  # 64 harmonics per core

    freq_d = nc.dram_tensor("freq", [HN, T], F, kind="ExternalInput")
    amp_d = nc.dram_tensor("amp", [HN, T], F, kind="ExternalInput")
    nba_d = nc.dram_tensor("nba", [NB // 2, T], F, kind="ExternalInput")
    nbb_d = nc.dram_tensor("nbb", [NB // 2, T], F, kind="ExternalInput")
    phiHz_d = nc.dram_tensor("phiHz", [128, NG], F, kind="ExternalInput")
    shiftM_d = nc.dram_tensor("shiftM", [128, 128], F, kind="ExternalInput")
    lhsT8_d = nc.dram_tensor("lhsT8", [128, 8], F, kind="ExternalInput")
    wlhsT_d = nc.dram_tensor("wlhsT", [128, 32], F, kind="ExternalInput")
    ecol_d = nc.dram_tensor("ecol", [128, 1], F, kind="ExternalInput")

    hc_d = nc.dram_tensor("hc_out", [8, TS], F, kind="ExternalOutput")
    nz_d = nc.dram_tensor("nz_out", [8, TS], F, kind="ExternalOutput")
    md_d = nc.dram_tensor("md_out", [2, 32, TC], F, kind="ExternalOutput")

    freq_r = freq_d[:, :].rearrange("h (tb t) -> (h tb) t", tb=NTB)   # [512, 8192]
    amp_r = amp_d[:, :].rearrange("h (tb t) -> (h tb) t", tb=NTB)
    nba_r = nba_d[:, :].rearrange("n (tb t) -> (n tb) t", tb=NTB)     # [128, 8192]
    nbb_r = nbb_d[:, :].rearrange("n (tb t) -> (n tb) t", tb=NTB)

    with TileContext(nc) as tc:
        with tc.tile_pool(name="big", bufs=1) as big, \
             tc.tile_pool(name="chunks", bufs=2) as ch, \
             tc.tile_pool(name="small", bufs=1) as sm, \
             tc.tile_pool(name="psum", bufs=2, space="PSUM") as pp:

            # constants
            lhsT8 = sm.tile([128, 8], F)
            nc.sync.dma_start(out=lhsT8, in_=lhsT8_d[:, :])
            shiftM = sm.tile([128, 128], F)
            nc.sync.dma_start(out=shiftM, in_=shiftM_d[:, :])
            phiHz = sm.tile([128, NG], F)
            nc.sync.dma_start(out=phiHz, in_=phiHz_d[:, :])
            wlhsT = sm.tile([128, 32], F)
            nc.sync.dma_start(out=wlhsT, in_=wlhsT_d[:, :])
            ecol = sm.tile([128, 1], F)
            nc.sync.dma_start(out=ecol, in_=ecol_d[:, :])
            zcol = sm.tile([128, 1], F)
            nc.vector.memset(zcol, 0.0)

            hc_sb = big.tile([8, TS], F)               # hc accumulator (pair sums)
            phase = [big.tile([128, TS], F, tag=f"phase{i}", name=f"phase{i}") for i in range(2)]
            staging = [big.tile([128, TC], F, tag=f"stage{i}", name=f"stage{i}") for i in range(2)]
            bias_sin = [sm.tile([128, 1], F, tag=f"bs{g}", name=f"bs{g}") for g in range(NG)]
            bias_qf = [sm.tile([128, 1], F, tag=f"bq{g}", name=f"bq{g}") for g in range(NG)]

            def l1(g):
                """mask+scan h-group g into phase[g % 2]; compute carry biases."""
                pb = phase[g % 2]
                prev_col = None
                for c in range(NCH):
                    ft = ch.tile([128, TC], F, tag="freq")
                    nc.sync.dma_start(
                        out=ft, in_=freq_r[g * 128:(g + 1) * 128,
                                           c * TC:(c + 1) * TC])
                    # masked f in-place: (f < SR/2) * f
                    nc.vector.scalar_tensor_tensor(
                        out=ft, in0=ft, scalar=float(SR / 2), in1=ft,
                        op0=mybir.AluOpType.is_lt, op1=mybir.AluOpType.mult)
                    seg = pb[:, c * TC:(c + 1) * TC]
                    nc.vector.tensor_tensor_scan(
                        out=seg, data0=ft, data1=ft,
                        initial=(zcol if prev_col is None else prev_col),
                        op0=mybir.AluOpType.add, op1=mybir.AluOpType.bypass)
                    red = sm.tile([128, 1], F, tag=f"red{c % 2}")
                    _round_cols(nc, sm, red, seg[:, TC - 1:TC], SR)
                    prev_col = red
                # slice carries: shiftM.T @ totals (totals = prev_col, reduced)
                cps = pp.tile([128, 1], F, tag="md_ps", bufs=1, name="cps")
                nc.tensor.matmul(cps, shiftM, prev_col, start=True, stop=True)
                csb = sm.tile([128, 1], F, tag="carry_sb")
                nc.scalar.copy(out=csb, in_=cps)
                cred = sm.tile([128, 1], F, tag="carry_red")
                _round_cols(nc, sm, cred, csb, SR)
                cb = sm.tile([128, 1], F, tag="cb")
                nc.vector.tensor_add(out=cb, in0=cred, in1=phiHz[:, g:g + 1])
                nc.vector.tensor_scalar(out=bias_sin[g], in0=cb, scalar1=K,
                                        scalar2=None, op0=mybir.AluOpType.mult)
                nc.vector.tensor_scalar(out=bias_qf[g], in0=cb, scalar1=INV_SR,
                                        scalar2=None, op0=mybir.AluOpType.mult)

            def l2_pair(pair_idx, gs):
                """consume phase bufs for groups gs (len 2); accumulate hc."""
                for c in range(NCH):
                    ps = pp.tile([8, TC], F, tag="hc_ps", bufs=2, name="ps")
                    for i, g in enumerate(gs):
                        pb = phase[g % 2]
                        seg = pb[:, c * TC:(c + 1) * TC]
                        qf = ch.tile([128, TC], F, tag="qf")
                        # qf = phase/SR + carry_term/SR
                        nc.scalar.activation(
                            out=qf, in_=seg,
                            func=mybir.ActivationFunctionType.Identity,
                            scale=INV_SR, bias=bias_qf[g])
                        # rnd = round(qf) in-place (Pool, 1-input)
                        nc.gpsimd.tensor_scalar(
                            out=qf, in0=qf, scalar1=RC, scalar2=RC,
                            op0=mybir.AluOpType.add,
                            op1=mybir.AluOpType.subtract)
                        # m = phase - SR*rnd  (in-place on qf)
                        nc.vector.scalar_tensor_tensor(
                            out=qf, in0=qf, scalar=-SR, in1=seg,
                            op0=mybir.AluOpType.mult, op1=mybir.AluOpType.add)
                        cosv = ch.tile([128, TC], F, tag="cos")
                        nc.scalar.activation(
                            out=cosv, in_=qf,
                            func=mybir.ActivationFunctionType.Sin,
                            scale=K, bias=bias_sin[g])
                        if g == 0:
                            half, cl = divmod(c, NCH // 2)
                            nc.sync.dma_start(
                                out=staging[half][cl * 32:(cl + 1) * 32, :],
                                in_=cosv[0:32, :])
                        at = ch.tile([128, TC], F, tag="amp")
                        nc.sync.dma_start(
                            out=at, in_=amp_r[g * 128:(g + 1) * 128,
                                              c * TC:(c + 1) * TC])
                        # prod in-place on cosv (Pool 2-input)
                        nc.gpsimd.tensor_mul(out=cosv, in0=cosv, in1=at)
                        for s in range(TC // 512):
                            nc.tensor.matmul(
                                ps[:, s * 512:(s + 1) * 512], lhsT8,
                                cosv[:, s * 512:(s + 1) * 512],
                                start=(i == 0), stop=(i == len(gs) - 1))
                    dst = hc_sb[:, c * TC:(c + 1) * TC]
                    if pair_idx == 0:
                        nc.scalar.copy(out=dst, in_=ps)
                    else:
                        nc.vector.tensor_add(out=dst, in0=dst, in1=ps)

            l1(0)
            l1(1)
            l2_pair(0, [0, 1])
            l1(2)
            l1(3)
            l2_pair(1, [2, 3])
            nc.sync.dma_start(out=hc_d[:, :], in_=hc_sb)

            # ---- noise path (g-independent) ----
            for c in range(NCH):
                bt = ch.tile([128, TC], F, tag="bands")
                nc.sync.dma_start(out=bt, in_=nbb_r[:, c * TC:(c + 1) * TC])
                atn = ch.tile([128, TC], F, tag="nba")
                nc.sync.dma_start(out=atn, in_=nba_r[:, c * TC:(c + 1) * TC])
                nc.vector.tensor_mul(out=bt, in0=bt, in1=atn)
                nps = pp.tile([8, TC], F, tag="nz_ps", bufs=1, name="nps")
                for s in range(TC // 512):
                    nc.tensor.matmul(nps[:, s * 512:(s + 1) * 512], lhsT8,
                                     bt[:, s * 512:(s + 1) * 512],
                                     start=True, stop=True)
                ncp = ch.tile([8, TC], F, tag="nz_sb")
                nc.scalar.copy(out=ncp, in_=nps)
                nc.sync.dma_start(out=nz_d[:, c * TC:(c + 1) * TC], in_=ncp)

            # ---- modulator path on staging tiles (harmonics 0..3) ----
            for half in range(2):
                st = staging[half]
                y = ch.tile([128, TC], F, tag="md_y")
                nc.scalar.mul(out=y, in_=st, mul=0.99)
                y2 = ch.tile([128, TC], F, tag="md_y2")
                nc.vector.tensor_mul(out=y2, in0=y, in1=y)
                nc.scalar.activation(out=y2, in_=y2,
                                     func=mybir.ActivationFunctionType.Sqrt,
                                     scale=-1.0, bias=1.0)
                nc.vector.reciprocal(out=y2, in_=y2)
                nc.vector.tensor_mul(out=y2, in0=y, in1=y2)
                nc.scalar.activation(out=y2, in_=y2,
                                     func=mybir.ActivationFunctionType.Arctan)
                nc.scalar.activation(out=y2, in_=y2,
                                     func=mybir.ActivationFunctionType.Abs,
                                     scale=float(2.0 / np.pi))
                nc.scalar.activation(out=y2, in_=y2,
                                     func=mybir.ActivationFunctionType.Ln)
                nc.vector.tensor_scalar_mul(out=y2, in0=y2, scalar1=ecol)
                nc.scalar.activation(out=y2, in_=y2,
                                     func=mybir.ActivationFunctionType.Exp)
                mps = pp.tile([32, TC], F, tag="md_ps", bufs=1, name="mps")
                for s in range(TC // 512):
                    nc.tensor.matmul(mps[:, s * 512:(s + 1) * 512], wlhsT,
                                     y2[:, s * 512:(s + 1) * 512],
                                     start=True, stop=True)
                mcp = ch.tile([32, TC], F, tag="md_sb")
                nc.scalar.copy(out=mcp, in_=mps)
                nc.sync.dma_start(out=md_d[half, :, :], in_=mcp)

    _split_multiwaits(nc)
    return nc


def kernel(**inputs):
    hf = np.ascontiguousarray(np.asarray(inputs["harmonic_frequencies"], np.float32))
    ha = np.ascontiguousarray(np.asarray(inputs["harmonic_amplitudes"], np.float32))
    nba = np.ascontiguousarray(np.asarray(inputs["noisebank_amplitudes"], np.float32))
    nbe = np.asarray(inputs["noisebank_mod_exponents"], np.float32)
    nbw = np.asarray(inputs["noisebank_mod_weights"], np.float32)
    pg = np.asarray(inputs["pulse_noise_gain"], np.float32)
    fg = np.asarray(inputs["flow_noise_gain"], np.float32)
    ip = np.asarray(inputs["initial_phase"], np.float32)
    nbands = np.ascontiguousarray(np.asarray(inputs["noise_bands"], np.float32))

    if "nc" not in _CACHE:
        _CACHE["nc"] = _build()
    nc = _CACHE["nc"]

    # host-side constant matrices (core-independent)
    p = np.arange(128)
    tb_p = p % 8
    lhsT8 = (tb_p[:, None] == np.arange(8)[None, :]).astype(np.float32)
    shiftM = ((p[:, None] // 8 == p[None, :] // 8) &
              (tb_p[:, None] < tb_p[None, :])).astype(np.float32)
    m_p = (p % 32) // 8           # modulator index per staging partition
    cl_p = p // 32                # chunk-local index per staging partition
    jj = np.arange(32)
    ind_mod = ((cl_p[:, None] == jj[None, :] // 8) &
               (tb_p[:, None] == jj[None, :] % 8)).astype(np.float32)

    in_maps = []
    for core in range(8):
        b, j = divmod(core, 2)
        hs = slice(j * 64, j * 64 + 64)
        ns = slice(j * 16, j * 16 + 16)
        # phiHz: (initial_phase + pi/2)/K per (h_local within group g, tb)
        iphz = ((ip[b, hs, 0].astype(np.float64) + np.pi / 2) / K).astype(np.float32)
        phiHz = np.zeros((128, NG), np.float32)
        for g in range(NG):
            phiHz[:, g] = np.repeat(iphz[g * HG:(g + 1) * HG], 8)
        wl = (ind_mod * nbw[b, m_p, 0][:, None]).astype(np.float32)
        ecol = nbe[b, m_p, 0].astype(np.float32).reshape(128, 1)
        in_maps.append(dict(
            freq=hf[b, hs], amp=ha[b, hs], nba=nba[b, ns], nbb=nbands[ns],
            phiHz=phiHz, shiftM=shiftM, lhsT8=lhsT8, wlhsT=wl, ecol=ecol))

    res = run_bass_kernel_spmd(nc, in_maps, core_ids=list(range(8)))
    outs = res.results

    # host combine
    out = np.empty((B, 1, T), np.float32)
    for b in range(B):
        r0, r1 = outs[2 * b], outs[2 * b + 1]
        hc = (r0["hc_out"].reshape(T) + r1["hc_out"].reshape(T))
        noise = (r0["nz_out"].reshape(T) + r1["nz_out"].reshape(T))
        # mod: md_out[half, j', tl], j' = c_local*8 + tb, t = tb*8192 + c*1024 + tl
        md = r0["md_out"].reshape(2, 4, 8, TC)          # [half, c_local, tb, tl]
        msum = np.ascontiguousarray(md.transpose(2, 0, 1, 3)).reshape(T)
        pgb = pg[b, 0, 0]; fgb = fg[b, 0, 0]
        tg = (pgb + fgb) * np.float32(0.7)
        out[b, 0] = (hc + msum * noise * pgb + hc * noise * tg
                     + noise * fgb * np.float32(0.3))
    return out



# revision 8
# speedup vs baseline: 2.2534x; 2.2534x over previous
"""HarmonicNoiseSynth Trainium2 kernel (v2).

Sharding: 8 cores = 4 batches x 2 harmonic halves (64 harmonics each); each
core also handles 16 of the 32 noise bands; every core runs the modulator
path on its first 4 local harmonics but the host only consumes it from j==0
cores (where those are the true modulators, harmonics 0..3).

Wire format (host-quantized to cut the ~30 MB/s axon transfer 3x):
  freq  -> uint16 phase units q = round(f * 65536/48000); phase quantization
           errors are independent per sample so the phase error random-walks:
           sigma ~ (2pi/65536)*sqrt(T)*0.29 ~ 5e-3 rad. Negligible.
  amp   -> uint8 (x255), dequant via SWDGE DMA cast to bf16 + 1/256 in lhsT,
           with a 256/255 host-side correction.
  nba   -> uint8 (x255), bands -> bf16.

Device pipeline per [128, 4096] half-slice (partition p = h_local*8 + tb,
tb = time slice of 8192; free dim = time within slice):
  1. scan (DVE): cumsum of q in fp32 (exact: chunk sums < 2^24), int32 out,
     initial = host-computed (phi0 + carry) mod 65536 per 1024-chunk.
  2. AND 0xFFFF (DVE): phase mod 65536 (per-element range reduction).
  3. Sin (ACT): cos via Sin(2pi/65536 * m - pi) = -cos(theta); the -1 is
     folded into lhsT. bf16 out.
  4. amp mul (POOL, bf16); 5. PE matmul partial sums, accumulated across the
     4 harmonic groups in PSUM ([64, 4096] tile: rows 0-7/8-15 hc/nz half 0,
     rows 32-39/40-47 hc/nz half 1).
The modulator path runs at the end, reusing the PSUM tile after evacuation;
|arcsin(0.99 c)|^e = Exp(e * Ln((2/pi)*Arctan(y*Rsqrt(1-y^2)))), y=Abs(.99c)
with the abs/scale/exponent multiplies folded into ACT affine slots.
"""
import sys

sys.path.insert(0, "/opt/trn_rl_repo")

import numpy as np
import ml_dtypes

import concourse.bass as bass
import concourse.mybir as mybir
from concourse.tile import TileContext
from concourse.bass_utils import run_bass_kernel_spmd

F = mybir.dt.float32
BF = mybir.dt.bfloat16
U8 = mybir.dt.uint8
U16 = mybir.dt.uint16
I32 = mybir.dt.int32

SR = 48000.0
B, H, NB, T = 4, 128, 32, 65536
NTB = 8          # time slices on partitions
TS = T // NTB    # 8192 per slice
TC = 1024        # scan chunk columns (cumsum stays < 2^24: exact fp32)
HB = 4096        # half-slice processed per pipeline step
NG = 4           # h-groups of 16 harmonics
HG = 16
MODQ = 65536.0
SCALE_Q = float(MODQ / SR)
KP = float(2.0 * np.pi / MODQ)

_CACHE = {}


def _split_multiwaits(nc):
    """This walrus build supports ONE sync wait per instruction; hoist extras
    onto single-wait NoOps inserted before the offending instruction."""
    ctr = 0
    for f in nc.m.functions:
        for bb in f.blocks:
            insts = list(bb.instructions)
            if not any(i.sync_info is not None and len(i.sync_info.on_wait) > 1
                       for i in insts):
                continue
            new = []
            for inst in insts:
                si = inst.sync_info
                if si is not None and len(si.on_wait) > 1:
                    waits = list(si.on_wait)
                    for w in waits[:-1]:
                        ctr += 1
                        nop = mybir.InstNoOp(name=f"mwsplit_{ctr}",
                                             engine=inst.engine)
                        nop.sync_info = mybir.SyncInfo(on_wait=[w], on_update=[])
                        new.append(nop)
                    inst.sync_info = mybir.SyncInfo(on_wait=[waits[-1]],
                                                    on_update=list(si.on_update))
                new.append(inst)
            bb.instructions = new
    return ctr


def _build():
    nc = bass.Bass("TRN2")
    HN = H // 2  # 64 harmonics per core

    q_d = nc.dram_tensor("q", [HN, T], U16, kind="ExternalInput")
    amp_d = nc.dram_tensor("amp", [HN, T], U8, kind="ExternalInput")
    nba_d = nc.dram_tensor("nba", [NB // 2, T], U8, kind="ExternalInput")
    nbb_d = nc.dram_tensor("nbb", [NB // 2, T], BF, kind="ExternalInput")
    init_d = nc.dram_tensor("init", [128, 32], F, kind="ExternalInput")
    lhsT_d = nc.dram_tensor("lhsT", [128, 64], BF, kind="ExternalInput")
    wlhsT_d = nc.dram_tensor("wlhsT", [128, 32], BF, kind="ExternalInput")
    ecol_d = nc.dram_tensor("ecol", [128, 1], F, kind="ExternalInput")

    out_d = nc.dram_tensor("out", [16, TS], F, kind="ExternalOutput")
    md_d = nc.dram_tensor("md_out", [2, 32, TC], F, kind="ExternalOutput")

    q_r = q_d[:, :].rearrange("h (tb t) -> (h tb) t", tb=NTB)      # [512, 8192]
    amp_r = amp_d[:, :].rearrange("h (tb t) -> (h tb) t", tb=NTB)
    nba_r = nba_d[:, :].rearrange("n (tb t) -> (n tb) t", tb=NTB)  # [128, 8192]
    nbb_r = nbb_d[:, :].rearrange("n (tb t) -> (n tb) t", tb=NTB)

    with TileContext(nc) as tc:
        with tc.tile_pool(name="sm", bufs=1) as sm, \
             tc.tile_pool(name="st", bufs=2) as st, \
             tc.tile_pool(name="pp", bufs=1, space="PSUM") as pp:

            lhsT = sm.tile([128, 64], BF)
            nc.sync.dma_start(out=lhsT, in_=lhsT_d[:, :])
            wlhsT = sm.tile([128, 32], BF)
            nc.sync.dma_start(out=wlhsT, in_=wlhsT_d[:, :])
            ecol = sm.tile([128, 1], F)
            nc.sync.dma_start(out=ecol, in_=ecol_d[:, :])
            init_sb = sm.tile([128, 32], F)
            nc.sync.dma_start(out=init_sb, in_=init_d[:, :])
            bsin = sm.tile([128, 1], F)
            nc.vector.memset(bsin, -np.pi)
            bone = sm.tile([128, 1], F)
            nc.vector.memset(bone, 1.0)
            stg = [sm.tile([128, TC], BF, tag=f"stg{i}", name=f"stg{i}")
                   for i in range(2)]
            # rows 0-7 hold hc, rows 32-39 hold nz (partition-aligned w/ psum)
            hcnz = [sm.tile([40, HB], F, tag=f"hcnz{i}", name=f"hcnz{i}")
                    for i in range(2)]

            # PSUM (matmul out base partition must be 0/32/64): rows 0-7 hc,
            # rows 32-39 nz for the current half; halves run sequentially.
            ps = pp.tile([64, HB], F, name="ps")

            for h5 in range(2):
                cols = slice(h5 * HB, (h5 + 1) * HB)
                for g in range(NG):
                    col0 = g * 8 + h5 * 4
                    rows = slice(g * 128, (g + 1) * 128)
                    qt = st.tile([128, HB], U16, tag="qt")
                    nc.sync.dma_start(out=qt, in_=q_r[rows, cols])
                    at = st.tile([128, HB], BF, tag="at")
                    nc.gpsimd.dma_start(out=at, in_=amp_r[rows, cols])
                    y = st.tile([128, HB], I32, tag="y")
                    for c in range(HB // TC):
                        nc.vector.tensor_tensor_scan(
                            out=y[:, c * TC:(c + 1) * TC],
                            data0=qt[:, c * TC:(c + 1) * TC],
                            data1=qt[:, c * TC:(c + 1) * TC],
                            initial=init_sb[:, col0 + c:col0 + c + 1],
                            op0=mybir.AluOpType.add,
                            op1=mybir.AluOpType.bypass)
                    nc.vector.tensor_scalar(out=y, in0=y, scalar1=0xFFFF,
                                            scalar2=None,
                                            op0=mybir.AluOpType.bitwise_and)
                    cosb = st.tile([128, HB], BF, tag="cosb")
                    nc.scalar.activation(out=cosb, in_=y,
                                         func=mybir.ActivationFunctionType.Sin,
                                         scale=KP, bias=bsin)
                    if g == 0:
                        for cl in range(4):
                            nc.sync.dma_start(
                                out=stg[h5][cl * 32:(cl + 1) * 32, :],
                                in_=cosb[0:32, cl * TC:(cl + 1) * TC])
                    nc.gpsimd.tensor_mul(out=cosb, in0=cosb, in1=at)
                    for s in range(HB // 512):
                        nc.tensor.matmul(
                            ps[0:8, s * 512:(s + 1) * 512],
                            lhsT[:, 0:8],
                            cosb[:, s * 512:(s + 1) * 512],
                            start=(g == 0), stop=(g == NG - 1))

                # noise: 16 bands x 8 tb on partitions; one mul + matmuls
                bt = st.tile([128, HB], BF, tag="bt", bufs=1)
                nc.sync.dma_start(out=bt, in_=nbb_r[:, cols])
                an = st.tile([128, HB], BF, tag="an", bufs=1)
                nc.gpsimd.dma_start(out=an, in_=nba_r[:, cols])
                nc.vector.tensor_mul(out=bt, in0=bt, in1=an)
                for s in range(HB // 512):
                    nc.tensor.matmul(ps[32:40, s * 512:(s + 1) * 512],
                                     lhsT[:, 32:40],
                                     bt[:, s * 512:(s + 1) * 512],
                                     start=True, stop=True)

                # evacuate hc (psum rows 0-7) + nz (rows 32-39)
                nc.scalar.copy(out=hcnz[h5][0:8, :], in_=ps[0:8, :])
                nc.scalar.copy(out=hcnz[h5][32:40, :], in_=ps[32:40, :])
                nc.sync.dma_start(out=out_d[0:8, h5 * HB:(h5 + 1) * HB],
                                  in_=hcnz[h5][0:8, :])
                nc.sync.dma_start(out=out_d[8:16, h5 * HB:(h5 + 1) * HB],
                                  in_=hcnz[h5][32:40, :])

            # ---- modulator path (staged -cos of local harmonics 0..3) ----
            ys, y2s = [], []
            for h5 in range(2):
                yv = sm.tile([128, TC], F, tag=f"my{h5}", name=f"my{h5}")
                nc.scalar.activation(out=yv, in_=stg[h5],
                                     func=mybir.ActivationFunctionType.Abs,
                                     scale=0.99)
                ys.append(yv)
            for h5 in range(2):
                y2 = sm.tile([128, TC], F, tag=f"my2{h5}", name=f"my2{h5}")
                nc.vector.tensor_mul(out=y2, in0=ys[h5], in1=ys[h5])
                y2s.append(y2)
            for h5 in range(2):   # s = sqrt(1 - y^2)
                nc.scalar.activation(out=y2s[h5], in_=y2s[h5],
                                     func=mybir.ActivationFunctionType.Sqrt,
                                     scale=-1.0, bias=bone)
            for h5 in range(2):   # r = 1/s
                nc.vector.reciprocal(out=y2s[h5], in_=y2s[h5])
            for h5 in range(2):   # t = y * r  (= tan(arcsin y))
                nc.vector.tensor_mul(out=ys[h5], in0=ys[h5], in1=y2s[h5])
            for h5 in range(2):
                nc.scalar.activation(out=ys[h5], in_=ys[h5],
                                     func=mybir.ActivationFunctionType.Arctan)
            for h5 in range(2):   # l = ln((2/pi) * arctan)
                nc.scalar.activation(out=ys[h5], in_=ys[h5],
                                     func=mybir.ActivationFunctionType.Ln,
                                     scale=float(2.0 / np.pi))
            shp = []
            for h5 in range(2):   # shaped = exp(e * l), bf16 for the matmul
                sb = sm.tile([128, TC], BF, tag=f"msh{h5}", name=f"msh{h5}")
                nc.scalar.activation(out=sb, in_=ys[h5],
                                     func=mybir.ActivationFunctionType.Exp,
                                     scale=ecol)
                shp.append(sb)
            for h5 in range(2):   # reuse evacuated psum banks for md
                mps = ps[0:32, h5 * TC:(h5 + 1) * TC]
                for s in range(TC // 512):
                    nc.tensor.matmul(mps[:, s * 512:(s + 1) * 512], wlhsT,
                                     shp[h5][:, s * 512:(s + 1) * 512],
                                     start=True, stop=True)
                mcp = sm.tile([32, TC], F, tag=f"mcp{h5}", name=f"mcp{h5}")
                nc.scalar.copy(out=mcp, in_=mps)
                nc.sync.dma_start(out=md_d[h5, :, :], in_=mcp)

    _split_multiwaits(nc)
    return nc


def kernel(**inputs):
    hf = np.asarray(inputs["harmonic_frequencies"], np.float32)
    ha = np.asarray(inputs["harmonic_amplitudes"], np.float32)
    nbaf = np.asarray(inputs["noisebank_amplitudes"], np.float32)
    nbe = np.asarray(inputs["noisebank_mod_exponents"], np.float32)
    nbw = np.asarray(inputs["noisebank_mod_weights"], np.float32)
    pg = np.asarray(inputs["pulse_noise_gain"], np.float32)
    fg = np.asarray(inputs["flow_noise_gain"], np.float32)
    ip = np.asarray(inputs["initial_phase"], np.float32)
    nbands = np.asarray(inputs["noise_bands"], np.float32)

    if "nc" not in _CACHE:
        _CACHE["nc"] = _build()
    nc = _CACHE["nc"]

    # quantize (all fp32-path, no float64 temporaries on the big arrays)
    q16 = (hf * np.float32(SCALE_Q) + np.float32(0.5)).astype(np.uint16)
    amp8 = (ha * np.float32(255.0) + np.float32(0.5)).astype(np.uint8)
    nba8 = (nbaf * np.float32(255.0) + np.float32(0.5)).astype(np.uint8)
    bandsbf = nbands.astype(ml_dtypes.bfloat16)

    # exact per-chunk carries (int64) + initial phase, mod 65536
    cs = q16.reshape(B, H, NTB * TS // TC, TC).sum(axis=3, dtype=np.int64)
    excl = np.cumsum(cs, axis=2) - cs                       # [B,H,64]
    phi0q = ((ip[..., 0].astype(np.float64) + np.pi / 2)
             * (MODQ / (2.0 * np.pi)))                      # [B,H]
    vals = ((excl % 65536).astype(np.float64)
            + phi0q[:, :, None]) % MODQ                     # [B,H,64]

    p = np.arange(128)
    tbp = p % 8
    lhsT = np.zeros((128, 64), np.float32)
    for jj in range(8):
        sel = tbp == jj
        lhsT[sel, jj] = -1.0 / 256         # hc (sign undoes the -sin fold)
        lhsT[sel, 32 + jj] = 1.0 / 256     # nz
    lhsT = lhsT.astype(ml_dtypes.bfloat16)
    m_p = (p % 32) // 8
    cl_p = p // 32
    jj32 = np.arange(32)
    ind_mod = ((cl_p[:, None] == jj32[None, :] // 8) &
               (tbp[:, None] == jj32[None, :] % 8)).astype(np.float32)

    in_maps = []
    for core in range(8):
        b, j = divmod(core, 2)
        hs = slice(j * 64, j * 64 + 64)
        ns = slice(j * 16, j * 16 + 16)
        vb = vals[b, hs]                    # [64 h_local, 64 chunk-ordinal]
        init = np.empty((128, 32), np.float32)
        for g in range(NG):
            init[:, g * 8:(g + 1) * 8] = \
                vb[g * HG:(g + 1) * HG].reshape(128, 8)
        wl = (ind_mod * nbw[b, m_p, 0][:, None]).astype(ml_dtypes.bfloat16)
        ec = nbe[b, m_p, 0].astype(np.float32).reshape(128, 1)
        in_maps.append(dict(q=q16[b, hs], amp=amp8[b, hs], nba=nba8[b, ns],
                            nbb=bandsbf[ns], init=init, lhsT=lhsT,
                            wlhsT=wl, ecol=ec))

    res = run_bass_kernel_spmd(nc, in_maps, core_ids=list(range(8)))
    outs = res.results

    sc = np.float32(256.0 / 255.0)
    out = np.empty((B, 1, T), np.float32)
    for b in range(B):
        r0, r1 = outs[2 * b], outs[2 * b + 1]
        hc = (r0["out"][0:8].reshape(T) + r1["out"][0:8].reshape(T)) * sc
        noise = (r0["out"][8:16].reshape(T) + r1["out"][8:16].reshape(T)) * sc
        # md[half, j', tl]: j' = cl*8 + tb; t = tb*8192 + (half*4+cl)*1024 + tl
        md = r0["md_out"].reshape(2, 4, 8, TC)
        msum = np.ascontiguousarray(md.transpose(2, 0, 1, 3)).reshape(T)
        pgb = pg[b, 0, 0]
        fgb = fg[b, 0, 0]
        tg = (pgb + fgb) * np.float32(0.7)
        out[b, 0] = (hc + msum * noise * pgb + hc * noise * tg
                     + noise * fgb * np.float32(0.3))
    return out


# revision 18
# speedup vs baseline: 2.3371x; 1.0372x over previous
"""HarmonicNoiseSynth Trainium2 kernel (v2).

Sharding: 8 cores = 4 batches x 2 harmonic halves (64 harmonics each); each
core also handles 16 of the 32 noise bands; every core runs the modulator
path on its first 4 local harmonics but the host only consumes it from j==0
cores (where those are the true modulators, harmonics 0..3).

Wire format (host-quantized to cut the ~30 MB/s axon transfer 3x):
  freq  -> uint16 phase units q = round(f * 65536/48000); phase quantization
           errors are independent per sample so the phase error random-walks:
           sigma ~ (2pi/65536)*sqrt(T)*0.29 ~ 5e-3 rad. Negligible.
  amp   -> uint8 (x255), dequant via SWDGE DMA cast to bf16 + 1/256 in lhsT,
           with a 256/255 host-side correction.
  nba   -> uint8 (x255), bands -> bf16.

Device pipeline per [128, 4096] half-slice (partition p = h_local*8 + tb,
tb = time slice of 8192; free dim = time within slice):
  1. scan (DVE): cumsum of q in fp32 (exact: chunk sums < 2^24), int32 out,
     initial = host-computed (phi0 + carry) mod 65536 per 1024-chunk.
  2. AND 0xFFFF (DVE): phase mod 65536 (per-element range reduction).
  3. Sin (ACT): cos via Sin(2pi/65536 * m - pi) = -cos(theta); the -1 is
     folded into lhsT. bf16 out.
  4. amp mul (POOL, bf16); 5. PE matmul partial sums, accumulated across the
     4 harmonic groups in PSUM ([64, 4096] tile: rows 0-7/8-15 hc/nz half 0,
     rows 32-39/40-47 hc/nz half 1).
The modulator path runs at the end, reusing the PSUM tile after evacuation;
|arcsin(0.99 c)|^e = Exp(e * Ln((2/pi)*Arctan(y*Rsqrt(1-y^2)))), y=Abs(.99c)
with the abs/scale/exponent multiplies folded into ACT affine slots.
"""
import sys

sys.path.insert(0, "/opt/trn_rl_repo")

import numpy as np
import ml_dtypes

import concourse.bass as bass
import concourse.mybir as mybir
from concourse.tile import TileContext
from concourse.bass_utils import run_bass_kernel_spmd

F = mybir.dt.float32
BF = mybir.dt.bfloat16
U8 = mybir.dt.uint8
U16 = mybir.dt.uint16
I32 = mybir.dt.int32

SR = 48000.0
B, H, NB, T = 4, 128, 32, 65536
NTB = 8          # time slices on partitions
TS = T // NTB    # 8192 per slice
TC = 1024        # scan chunk columns (cumsum stays < 2^24: exact fp32)
HB = 4096        # half-slice processed per pipeline step
NG = 4           # h-groups of 16 harmonics
HG = 16
MODQ = 65536.0
SCALE_Q = float(MODQ / SR)
KP = float(2.0 * np.pi / MODQ)

_CACHE = {}

# which (h5, g) half-slices run the amp*cos multiply on DVE (rest on Pool)
# and where the noise mul runs: tuned with the TimelineSim cost model.
MUL_DVE_STEPS = {0, 1, 2, 3, 4, 5, 6, 7}
NOISE_MUL_DVE = False


def _split_multiwaits(nc):
    """This walrus build supports ONE sync wait per instruction; hoist extras
    onto single-wait NoOps inserted before the offending instruction."""
    ctr = 0
    for f in nc.m.functions:
        for bb in f.blocks:
            insts = list(bb.instructions)
            if not any(i.sync_info is not None and len(i.sync_info.on_wait) > 1
                       for i in insts):
                continue
            new = []
            for inst in insts:
                si = inst.sync_info
                if si is not None and len(si.on_wait) > 1:
                    waits = list(si.on_wait)
                    for w in waits[:-1]:
                        ctr += 1
                        nop = mybir.InstNoOp(name=f"mwsplit_{ctr}",
                                             engine=inst.engine)
                        nop.sync_info = mybir.SyncInfo(on_wait=[w], on_update=[])
                        new.append(nop)
                    inst.sync_info = mybir.SyncInfo(on_wait=[waits[-1]],
                                                    on_update=list(si.on_update))
                new.append(inst)
            bb.instructions = new
    return ctr


def _build():
    nc = bass.Bass("TRN2")
    HN = H // 2  # 64 harmonics per core

    q_d = nc.dram_tensor("q", [HN, T], U16, kind="ExternalInput")
    amp_d = nc.dram_tensor("amp", [HN, T], U8, kind="ExternalInput")
    nba_d = nc.dram_tensor("nba", [NB // 2, T], U8, kind="ExternalInput")
    nbb_d = nc.dram_tensor("nbb", [NB // 2, T], BF, kind="ExternalInput")
    init_d = nc.dram_tensor("init", [128, 32], F, kind="ExternalInput")
    lhsT_d = nc.dram_tensor("lhsT", [128, 64], BF, kind="ExternalInput")
    wlhsT_d = nc.dram_tensor("wlhsT", [128, 32], BF, kind="ExternalInput")
    ecol_d = nc.dram_tensor("ecol", [128, 1], F, kind="ExternalInput")

    out_d = nc.dram_tensor("out", [16, TS], BF, kind="ExternalOutput")
    md_d = nc.dram_tensor("md_out", [2, 32, TC], BF, kind="ExternalOutput")

    q_r = q_d[:, :].rearrange("h (tb t) -> (h tb) t", tb=NTB)      # [512, 8192]
    amp_r = amp_d[:, :].rearrange("h (tb t) -> (h tb) t", tb=NTB)
    nba_r = nba_d[:, :].rearrange("n (tb t) -> (n tb) t", tb=NTB)  # [128, 8192]
    nbb_r = nbb_d[:, :].rearrange("n (tb t) -> (n tb) t", tb=NTB)

    with TileContext(nc) as tc:
        with tc.tile_pool(name="sm", bufs=1) as sm, \
             tc.tile_pool(name="st", bufs=2) as st, \
             tc.tile_pool(name="pp", bufs=1, space="PSUM") as pp:

            lhsT = sm.tile([128, 64], BF)
            nc.sync.dma_start(out=lhsT, in_=lhsT_d[:, :])
            wlhsT = sm.tile([128, 32], BF)
            nc.sync.dma_start(out=wlhsT, in_=wlhsT_d[:, :])
            ecol = sm.tile([128, 1], F)
            nc.sync.dma_start(out=ecol, in_=ecol_d[:, :])
            init_sb = sm.tile([128, 32], F)
            nc.sync.dma_start(out=init_sb, in_=init_d[:, :])
            bsin = sm.tile([128, 1], F)
            nc.vector.memset(bsin, -np.pi)
            bone = sm.tile([128, 1], F)
            nc.vector.memset(bone, 1.0)
            stg = [sm.tile([128, TC], BF, tag=f"stg{i}", name=f"stg{i}")
                   for i in range(2)]
            # rows 0-7 hold hc, rows 32-39 hold nz (partition-aligned w/ psum)
            hcnz = [sm.tile([40, HB], BF, tag=f"hcnz{i}", name=f"hcnz{i}")
                    for i in range(2)]

            # PSUM (matmul out base partition must be 0/32/64): rows 0-7 hc,
            # rows 32-39 nz for the current half; halves run sequentially.
            ps = pp.tile([64, HB], F, name="ps")

            for h5 in range(2):
                cols = slice(h5 * HB, (h5 + 1) * HB)
                for g in range(NG):
                    col0 = g * 8 + h5 * 4
                    rows = slice(g * 128, (g + 1) * 128)
                    qt = st.tile([128, HB], U16, tag="qt")
                    nc.sync.dma_start(out=qt, in_=q_r[rows, cols])
                    at = st.tile([128, HB], BF, tag="at")
                    nc.gpsimd.dma_start(out=at, in_=amp_r[rows, cols])
                    y = st.tile([128, HB], I32, tag="y")
                    for c in range(HB // TC):
                        nc.vector.tensor_tensor_scan(
                            out=y[:, c * TC:(c + 1) * TC],
                            data0=qt[:, c * TC:(c + 1) * TC],
                            data1=qt[:, c * TC:(c + 1) * TC],
                            initial=init_sb[:, col0 + c:col0 + c + 1],
                            op0=mybir.AluOpType.add,
                            op1=mybir.AluOpType.bypass)
                    nc.vector.tensor_scalar(out=y, in0=y, scalar1=0xFFFF,
                                            scalar2=None,
                                            op0=mybir.AluOpType.bitwise_and)
                    cosb = st.tile([128, HB], BF, tag="cosb")
                    nc.scalar.activation(out=cosb, in_=y,
                                         func=mybir.ActivationFunctionType.Sin,
                                         scale=KP, bias=bsin)
                    if g == 0:
                        for cl in range(4):
                            nc.sync.dma_start(
                                out=stg[h5][cl * 32:(cl + 1) * 32, :],
                                in_=cosb[0:32, cl * TC:(cl + 1) * TC])
                    if h5 * 4 + g in MUL_DVE_STEPS:
                        nc.vector.tensor_mul(out=cosb, in0=cosb, in1=at)
                    else:
                        nc.gpsimd.tensor_mul(out=cosb, in0=cosb, in1=at)
                    for s in range(HB // 512):
                        nc.tensor.matmul(
                            ps[0:8, s * 512:(s + 1) * 512],
                            lhsT[:, 0:8],
                            cosb[:, s * 512:(s + 1) * 512],
                            start=(g == 0), stop=(g == NG - 1))

                # noise: 16 bands x 8 tb on partitions; one mul + matmuls
                bt = st.tile([128, HB], BF, tag="bt", bufs=1)
                nc.sync.dma_start(out=bt, in_=nbb_r[:, cols])
                an = st.tile([128, HB], BF, tag="an", bufs=1)
                nc.gpsimd.dma_start(out=an, in_=nba_r[:, cols])
                if NOISE_MUL_DVE:
                    nc.vector.tensor_mul(out=bt, in0=bt, in1=an)
                else:
                    nc.gpsimd.tensor_mul(out=bt, in0=bt, in1=an)
                for s in range(HB // 512):
                    nc.tensor.matmul(ps[32:40, s * 512:(s + 1) * 512],
                                     lhsT[:, 32:40],
                                     bt[:, s * 512:(s + 1) * 512],
                                     start=True, stop=True)

                # evacuate hc (psum rows 0-7) + nz (rows 32-39)
                nc.scalar.copy(out=hcnz[h5][0:8, :], in_=ps[0:8, :])
                nc.scalar.copy(out=hcnz[h5][32:40, :], in_=ps[32:40, :])
                nc.sync.dma_start(out=out_d[0:8, h5 * HB:(h5 + 1) * HB],
                                  in_=hcnz[h5][0:8, :])
                nc.sync.dma_start(out=out_d[8:16, h5 * HB:(h5 + 1) * HB],
                                  in_=hcnz[h5][32:40, :])

            # ---- modulator path (staged -cos of local harmonics 0..3) ----
            ys, y2s = [], []
            for h5 in range(2):
                yv = sm.tile([128, TC], F, tag=f"my{h5}", name=f"my{h5}")
                nc.scalar.activation(out=yv, in_=stg[h5],
                                     func=mybir.ActivationFunctionType.Abs,
                                     scale=0.99)
                ys.append(yv)
            for h5 in range(2):
                y2 = sm.tile([128, TC], F, tag=f"my2{h5}", name=f"my2{h5}")
                nc.vector.tensor_mul(out=y2, in0=ys[h5], in1=ys[h5])
                y2s.append(y2)
            for h5 in range(2):   # s = sqrt(1 - y^2)
                nc.scalar.activation(out=y2s[h5], in_=y2s[h5],
                                     func=mybir.ActivationFunctionType.Sqrt,
                                     scale=-1.0, bias=bone)
            for h5 in range(2):   # r = 1/s
                nc.vector.reciprocal(out=y2s[h5], in_=y2s[h5])
            for h5 in range(2):   # t = y * r  (= tan(arcsin y))
                nc.vector.tensor_mul(out=ys[h5], in0=ys[h5], in1=y2s[h5])
            for h5 in range(2):
                nc.scalar.activation(out=ys[h5], in_=ys[h5],
                                     func=mybir.ActivationFunctionType.Arctan)
            for h5 in range(2):   # l = ln((2/pi) * arctan)
                nc.scalar.activation(out=ys[h5], in_=ys[h5],
                                     func=mybir.ActivationFunctionType.Ln,
                                     scale=float(2.0 / np.pi))
            shp = []
            for h5 in range(2):   # shaped = exp(e * l), bf16 for the matmul
                sb = sm.tile([128, TC], BF, tag=f"msh{h5}", name=f"msh{h5}")
                nc.scalar.activation(out=sb, in_=ys[h5],
                                     func=mybir.ActivationFunctionType.Exp,
                                     scale=ecol)
                shp.append(sb)
            for h5 in range(2):   # reuse evacuated psum banks for md
                mps = ps[0:32, h5 * TC:(h5 + 1) * TC]
                for s in range(TC // 512):
                    nc.tensor.matmul(mps[:, s * 512:(s + 1) * 512], wlhsT,
                                     shp[h5][:, s * 512:(s + 1) * 512],
                                     start=True, stop=True)
                mcp = sm.tile([32, TC], BF, tag=f"mcp{h5}", name=f"mcp{h5}")
                nc.scalar.copy(out=mcp, in_=mps)
                nc.sync.dma_start(out=md_d[h5, :, :], in_=mcp)

    _split_multiwaits(nc)
    return nc


def kernel(**inputs):
    hf = np.asarray(inputs["harmonic_frequencies"], np.float32)
    ha = np.asarray(inputs["harmonic_amplitudes"], np.float32)
    nbaf = np.asarray(inputs["noisebank_amplitudes"], np.float32)
    nbe = np.asarray(inputs["noisebank_mod_exponents"], np.float32)
    nbw = np.asarray(inputs["noisebank_mod_weights"], np.float32)
    pg = np.asarray(inputs["pulse_noise_gain"], np.float32)
    fg = np.asarray(inputs["flow_noise_gain"], np.float32)
    ip = np.asarray(inputs["initial_phase"], np.float32)
    nbands = np.asarray(inputs["noise_bands"], np.float32)

    if "nc" not in _CACHE:
        _CACHE["nc"] = _build()
    nc = _CACHE["nc"]

    # quantize (all fp32-path, no float64 temporaries on the big arrays);
    # numpy releases the GIL on large ufuncs, so run the three big
    # conversions in parallel.
    from concurrent.futures import ThreadPoolExecutor
    with ThreadPoolExecutor(3) as ex:
        fq = ex.submit(lambda: (hf * np.float32(SCALE_Q)
                                + np.float32(0.5)).astype(np.uint16))
        fa = ex.submit(lambda: (ha * np.float32(255.0)
                                + np.float32(0.5)).astype(np.uint8))
        fn = ex.submit(lambda: ((nbaf * np.float32(255.0)
                                 + np.float32(0.5)).astype(np.uint8),
                                nbands.astype(ml_dtypes.bfloat16)))
        q16 = fq.result()
        amp8 = fa.result()
        nba8, bandsbf = fn.result()

    # exact per-chunk carries (int64) + initial phase, mod 65536
    cs = q16.reshape(B, H, NTB * TS // TC, TC).sum(axis=3, dtype=np.int64)
    excl = np.cumsum(cs, axis=2) - cs                       # [B,H,64]
    phi0q = ((ip[..., 0].astype(np.float64) + np.pi / 2)
             * (MODQ / (2.0 * np.pi)))                      # [B,H]
    vals = ((excl % 65536).astype(np.float64)
            + phi0q[:, :, None]) % MODQ                     # [B,H,64]

    p = np.arange(128)
    tbp = p % 8
    lhsT = np.zeros((128, 64), np.float32)
    for jj in range(8):
        sel = tbp == jj
        lhsT[sel, jj] = -1.0 / 256         # hc (sign undoes the -sin fold)
        lhsT[sel, 32 + jj] = 1.0 / 256     # nz
    lhsT = lhsT.astype(ml_dtypes.bfloat16)
    m_p = (p % 32) // 8
    cl_p = p // 32
    jj32 = np.arange(32)
    ind_mod = ((cl_p[:, None] == jj32[None, :] // 8) &
               (tbp[:, None] == jj32[None, :] % 8)).astype(np.float32)

    in_maps = []
    for core in range(8):
        b, j = divmod(core, 2)
        hs = slice(j * 64, j * 64 + 64)
        ns = slice(j * 16, j * 16 + 16)
        vb = vals[b, hs]                    # [64 h_local, 64 chunk-ordinal]
        init = np.empty((128, 32), np.float32)
        for g in range(NG):
            init[:, g * 8:(g + 1) * 8] = \
                vb[g * HG:(g + 1) * HG].reshape(128, 8)
        wl = (ind_mod * nbw[b, m_p, 0][:, None]).astype(ml_dtypes.bfloat16)
        ec = nbe[b, m_p, 0].astype(np.float32).reshape(128, 1)
        in_maps.append(dict(q=q16[b, hs], amp=amp8[b, hs], nba=nba8[b, ns],
                            nbb=bandsbf[ns], init=init, lhsT=lhsT,
                            wlhsT=wl, ecol=ec))

    res = run_bass_kernel_spmd(nc, in_maps, core_ids=list(range(8)))
    outs = res.results

    sc = np.float32(256.0 / 255.0)
    out = np.empty((B, 1, T), np.float32)
    for b in range(B):
        r0, r1 = outs[2 * b], outs[2 * b + 1]
        o0 = r0["out"].astype(np.float32)
        o1 = r1["out"].astype(np.float32)
        hc = (o0[0:8].reshape(T) + o1[0:8].reshape(T)) * sc
        noise = (o0[8:16].reshape(T) + o1[8:16].reshape(T)) * sc
        # md[half, j', tl]: j' = cl*8 + tb; t = tb*8192 + (half*4+cl)*1024 + tl
        md = r0["md_out"].astype(np.float32).reshape(2, 4, 8, TC)
        msum = np.ascontiguousarray(md.transpose(2, 0, 1, 3)).reshape(T)
        pgb = pg[b, 0, 0]
        fgb = fg[b, 0, 0]
        tg = (pgb + fgb) * np.float32(0.7)
        out[b, 0] = (hc + msum * noise * pgb + hc * noise * tg
                     + noise * fgb * np.float32(0.3))
    return out


# revision 26
# speedup vs baseline: 2.5012x; 1.0702x over previous
"""HarmonicNoiseSynth Trainium2 kernel (v2).

Sharding: 8 cores = 4 batches x 2 harmonic halves (64 harmonics each); each
core also handles 16 of the 32 noise bands; every core runs the modulator
path on its first 4 local harmonics but the host only consumes it from j==0
cores (where those are the true modulators, harmonics 0..3).

Wire format (host-quantized to cut the ~30 MB/s axon transfer 3x):
  freq  -> uint16 phase units q = round(f * 65536/48000); phase quantization
           errors are independent per sample so the phase error random-walks:
           sigma ~ (2pi/65536)*sqrt(T)*0.29 ~ 5e-3 rad. Negligible.
  amp   -> uint8 (x255), dequant via SWDGE DMA cast to bf16 + 1/256 in lhsT,
           with a 256/255 host-side correction.
  nba   -> uint8 (x255), bands -> bf16.

Device pipeline per [128, 4096] half-slice (partition p = h_local*8 + tb,
tb = time slice of 8192; free dim = time within slice):
  1. scan (DVE): cumsum of q in fp32 (exact: chunk sums < 2^24), int32 out,
     initial = host-computed (phi0 + carry) mod 65536 per 1024-chunk.
  2. AND 0xFFFF (DVE): phase mod 65536 (per-element range reduction).
  3. Sin (ACT): cos via Sin(2pi/65536 * m - pi) = -cos(theta); the -1 is
     folded into lhsT. bf16 out.
  4. amp mul (POOL, bf16); 5. PE matmul partial sums, accumulated across the
     4 harmonic groups in PSUM ([64, 4096] tile: rows 0-7/8-15 hc/nz half 0,
     rows 32-39/40-47 hc/nz half 1).
The modulator path runs at the end, reusing the PSUM tile after evacuation;
|arcsin(0.99 c)|^e = Exp(e * Ln((2/pi)*Arctan(y*Rsqrt(1-y^2)))), y=Abs(.99c)
with the abs/scale/exponent multiplies folded into ACT affine slots.
"""
import sys

sys.path.insert(0, "/opt/trn_rl_repo")

import numpy as np
import ml_dtypes

import concourse.bass as bass
import concourse.mybir as mybir
from concourse.tile import TileContext
from concourse.bass_utils import run_bass_kernel_spmd

F = mybir.dt.float32
BF = mybir.dt.bfloat16
U8 = mybir.dt.uint8
U16 = mybir.dt.uint16
I32 = mybir.dt.int32

SR = 48000.0
B, H, NB, T = 4, 128, 32, 65536
NTB = 8          # time slices on partitions
TS = T // NTB    # 8192 per slice
TC = 1024        # scan chunk columns (cumsum stays < 2^24: exact fp32)
HB = 4096        # half-slice processed per pipeline step
NG = 4           # h-groups of 16 harmonics
HG = 16
MODQ = 65536.0
SCALE_Q = float(MODQ / SR)
KP = float(2.0 * np.pi / MODQ)

_CACHE = {}

# which (h5, g) half-slices run the amp*cos multiply on DVE (rest on Pool)
# and where the noise mul runs: tuned with the TimelineSim cost model.
MUL_DVE_STEPS = {0, 1, 2, 3, 4, 5, 6, 7}
NOISE_MUL_DVE = False


def _split_multiwaits(nc):
    """This walrus build supports ONE sync wait per instruction; hoist extras
    onto single-wait NoOps inserted before the offending instruction."""
    ctr = 0
    for f in nc.m.functions:
        for bb in f.blocks:
            insts = list(bb.instructions)
            if not any(i.sync_info is not None and len(i.sync_info.on_wait) > 1
                       for i in insts):
                continue
            new = []
            for inst in insts:
                si = inst.sync_info
                if si is not None and len(si.on_wait) > 1:
                    waits = list(si.on_wait)
                    for w in waits[:-1]:
                        ctr += 1
                        nop = mybir.InstNoOp(name=f"mwsplit_{ctr}",
                                             engine=inst.engine)
                        nop.sync_info = mybir.SyncInfo(on_wait=[w], on_update=[])
                        new.append(nop)
                    inst.sync_info = mybir.SyncInfo(on_wait=[waits[-1]],
                                                    on_update=list(si.on_update))
                new.append(inst)
            bb.instructions = new
    return ctr


def _build():
    nc = bass.Bass("TRN2")
    HN = H // 2  # 64 harmonics per core

    q_d = nc.dram_tensor("q", [HN, T], U16, kind="ExternalInput")
    amp_d = nc.dram_tensor("amp", [HN, T], U8, kind="ExternalInput")
    # noise: each core owns 4 unique bands and computes partials for ALL 4
    # batches (avoids shipping duplicate band data): nba rows = (batch, band)
    nba_d = nc.dram_tensor("nba", [16, T], U8, kind="ExternalInput")
    nbb_d = nc.dram_tensor("nbb", [4, T], BF, kind="ExternalInput")
    init_d = nc.dram_tensor("init", [128, 32], F, kind="ExternalInput")
    lhsT_d = nc.dram_tensor("lhsT", [128, 64], BF, kind="ExternalInput")
    wlhsT_d = nc.dram_tensor("wlhsT", [128, 32], BF, kind="ExternalInput")
    ecol_d = nc.dram_tensor("ecol", [128, 1], F, kind="ExternalInput")

    out_d = nc.dram_tensor("out", [40, TS], BF, kind="ExternalOutput")
    md_d = nc.dram_tensor("md_out", [2, 32, TC], BF, kind="ExternalOutput")

    q_r = q_d[:, :].rearrange("h (tb t) -> (h tb) t", tb=NTB)      # [512, 8192]
    amp_r = amp_d[:, :].rearrange("h (tb t) -> (h tb) t", tb=NTB)
    nba_r = nba_d[:, :].rearrange("n (tb t) -> (n tb) t", tb=NTB)  # [128, 8192]
    nbb_r = nbb_d[:, :].rearrange("n (tb t) -> (n tb) t", tb=NTB)  # [32, 8192]

    with TileContext(nc) as tc:
        with tc.tile_pool(name="sm", bufs=1) as sm, \
             tc.tile_pool(name="st", bufs=2) as st, \
             tc.tile_pool(name="pp", bufs=1, space="PSUM") as pp:

            lhsT = sm.tile([128, 64], BF)
            nc.sync.dma_start(out=lhsT, in_=lhsT_d[:, :])
            wlhsT = sm.tile([128, 32], BF)
            nc.sync.dma_start(out=wlhsT, in_=wlhsT_d[:, :])
            ecol = sm.tile([128, 1], F)
            nc.sync.dma_start(out=ecol, in_=ecol_d[:, :])
            init_sb = sm.tile([128, 32], F)
            nc.sync.dma_start(out=init_sb, in_=init_d[:, :])
            bsin = sm.tile([128, 1], F)
            nc.vector.memset(bsin, -np.pi)
            bone = sm.tile([128, 1], F)
            nc.vector.memset(bone, 1.0)
            stg = [sm.tile([128, TC], BF, tag=f"stg{i}", name=f"stg{i}")
                   for i in range(2)]
            # rows 0-7 hold hc, rows 32-63 hold nz (partition-aligned w/ psum)
            hcnz = [sm.tile([64, HB], BF, tag=f"hcnz{i}", name=f"hcnz{i}")
                    for i in range(2)]

            # PSUM (matmul out base partition must be 0/32/64): rows 0-7 hc,
            # rows 32-39 nz for the current half; halves run sequentially.
            ps = pp.tile([64, HB], F, name="ps")

            for h5 in range(2):
                cols = slice(h5 * HB, (h5 + 1) * HB)
                for g in range(NG):
                    col0 = g * 8 + h5 * 4
                    rows = slice(g * 128, (g + 1) * 128)
                    qt = st.tile([128, HB], U16, tag="qt")
                    nc.sync.dma_start(out=qt, in_=q_r[rows, cols])
                    at = st.tile([128, HB], BF, tag="at")
                    nc.gpsimd.dma_start(out=at, in_=amp_r[rows, cols])
                    y = st.tile([128, HB], I32, tag="y")
                    for c in range(HB // TC):
                        nc.vector.tensor_tensor_scan(
                            out=y[:, c * TC:(c + 1) * TC],
                            data0=qt[:, c * TC:(c + 1) * TC],
                            data1=qt[:, c * TC:(c + 1) * TC],
                            initial=init_sb[:, col0 + c:col0 + c + 1],
                            op0=mybir.AluOpType.add,
                            op1=mybir.AluOpType.bypass)
                    nc.vector.tensor_scalar(out=y, in0=y, scalar1=0xFFFF,
                                            scalar2=None,
                                            op0=mybir.AluOpType.bitwise_and)
                    cosb = st.tile([128, HB], BF, tag="cosb")
                    nc.scalar.activation(out=cosb, in_=y,
                                         func=mybir.ActivationFunctionType.Sin,
                                         scale=KP, bias=bsin)
                    if g == 0:
                        for cl in range(4):
                            nc.sync.dma_start(
                                out=stg[h5][cl * 32:(cl + 1) * 32, :],
                                in_=cosb[0:32, cl * TC:(cl + 1) * TC])
                    if h5 * 4 + g in MUL_DVE_STEPS:
                        nc.vector.tensor_mul(out=cosb, in0=cosb, in1=at)
                    else:
                        nc.gpsimd.tensor_mul(out=cosb, in0=cosb, in1=at)
                    for s in range(HB // 512):
                        nc.tensor.matmul(
                            ps[0:8, s * 512:(s + 1) * 512],
                            lhsT[:, 0:8],
                            cosb[:, s * 512:(s + 1) * 512],
                            start=(g == 0), stop=(g == NG - 1))

                # noise: partition (b*4 + n)*8 + tb; band rows replicated 4x
                bt = st.tile([128, HB], BF, tag="bt", bufs=1)
                for bb in range(4):
                    nc.sync.dma_start(out=bt[bb * 32:(bb + 1) * 32, :],
                                      in_=nbb_r[:, cols])
                an = st.tile([128, HB], BF, tag="an", bufs=1)
                nc.gpsimd.dma_start(out=an, in_=nba_r[:, cols])
                if NOISE_MUL_DVE:
                    nc.vector.tensor_mul(out=bt, in0=bt, in1=an)
                else:
                    nc.gpsimd.tensor_mul(out=bt, in0=bt, in1=an)
                for s in range(HB // 512):
                    nc.tensor.matmul(ps[32:64, s * 512:(s + 1) * 512],
                                     lhsT[:, 32:64],
                                     bt[:, s * 512:(s + 1) * 512],
                                     start=True, stop=True)

                # evacuate hc (psum rows 0-7) + nz partials (rows 32-63)
                nc.scalar.copy(out=hcnz[h5][0:8, :], in_=ps[0:8, :])
                nc.scalar.copy(out=hcnz[h5][32:64, :], in_=ps[32:64, :])
                nc.sync.dma_start(out=out_d[0:8, h5 * HB:(h5 + 1) * HB],
                                  in_=hcnz[h5][0:8, :])
                nc.sync.dma_start(out=out_d[8:40, h5 * HB:(h5 + 1) * HB],
                                  in_=hcnz[h5][32:64, :])

            # ---- modulator path (staged -cos of local harmonics 0..3) ----
            ys, y2s = [], []
            for h5 in range(2):
                yv = sm.tile([128, TC], F, tag=f"my{h5}", name=f"my{h5}")
                nc.scalar.activation(out=yv, in_=stg[h5],
                                     func=mybir.ActivationFunctionType.Abs,
                                     scale=0.99)
                ys.append(yv)
            for h5 in range(2):
                y2 = sm.tile([128, TC], F, tag=f"my2{h5}", name=f"my2{h5}")
                nc.vector.tensor_mul(out=y2, in0=ys[h5], in1=ys[h5])
                y2s.append(y2)
            for h5 in range(2):   # s = sqrt(1 - y^2)
                nc.scalar.activation(out=y2s[h5], in_=y2s[h5],
                                     func=mybir.ActivationFunctionType.Sqrt,
                                     scale=-1.0, bias=bone)
            for h5 in range(2):   # r = 1/s
                nc.vector.reciprocal(out=y2s[h5], in_=y2s[h5])
            for h5 in range(2):   # t = y * r  (= tan(arcsin y))
                nc.vector.tensor_mul(out=ys[h5], in0=ys[h5], in1=y2s[h5])
            for h5 in range(2):
                nc.scalar.activation(out=ys[h5], in_=ys[h5],
                                     func=mybir.ActivationFunctionType.Arctan)
            for h5 in range(2):   # l = ln((2/pi) * arctan)
                nc.scalar.activation(out=ys[h5], in_=ys[h5],
                                     func=mybir.ActivationFunctionType.Ln,
                                     scale=float(2.0 / np.pi))
            shp = []
            for h5 in range(2):   # shaped = exp(e * l), bf16 for the matmul
                sb = sm.tile([128, TC], BF, tag=f"msh{h5}", name=f"msh{h5}")
                nc.scalar.activation(out=sb, in_=ys[h5],
                                     func=mybir.ActivationFunctionType.Exp,
                                     scale=ecol)
                shp.append(sb)
            for h5 in range(2):   # reuse evacuated psum banks for md
                mps = ps[0:32, h5 * TC:(h5 + 1) * TC]
                for s in range(TC // 512):
                    nc.tensor.matmul(mps[:, s * 512:(s + 1) * 512], wlhsT,
                                     shp[h5][:, s * 512:(s + 1) * 512],
                                     start=True, stop=True)
                mcp = sm.tile([32, TC], BF, tag=f"mcp{h5}", name=f"mcp{h5}")
                nc.scalar.copy(out=mcp, in_=mps)
                nc.sync.dma_start(out=md_d[h5, :, :], in_=mcp)

    _split_multiwaits(nc)
    return nc


def kernel(**inputs):
    hf = np.asarray(inputs["harmonic_frequencies"], np.float32)
    ha = np.asarray(inputs["harmonic_amplitudes"], np.float32)
    nbaf = np.asarray(inputs["noisebank_amplitudes"], np.float32)
    nbe = np.asarray(inputs["noisebank_mod_exponents"], np.float32)
    nbw = np.asarray(inputs["noisebank_mod_weights"], np.float32)
    pg = np.asarray(inputs["pulse_noise_gain"], np.float32)
    fg = np.asarray(inputs["flow_noise_gain"], np.float32)
    ip = np.asarray(inputs["initial_phase"], np.float32)
    nbands = np.asarray(inputs["noise_bands"], np.float32)

    if "nc" not in _CACHE:
        _CACHE["nc"] = _build()
    nc = _CACHE["nc"]

    # quantize (all fp32-path, no float64 temporaries on the big arrays);
    # numpy releases the GIL on large ufuncs, so run the three big
    # conversions in parallel.
    from concurrent.futures import ThreadPoolExecutor
    with ThreadPoolExecutor(3) as ex:
        fq = ex.submit(lambda: (hf * np.float32(SCALE_Q)
                                + np.float32(0.5)).astype(np.uint16))
        fa = ex.submit(lambda: (ha * np.float32(255.0)
                                + np.float32(0.5)).astype(np.uint8))
        fn = ex.submit(lambda: ((nbaf * np.float32(255.0)
                                 + np.float32(0.5)).astype(np.uint8),
                                nbands.astype(ml_dtypes.bfloat16)))
        q16 = fq.result()
        amp8 = fa.result()
        nba8, bandsbf = fn.result()

    # exact per-chunk carries (int64) + initial phase, mod 65536
    cs = q16.reshape(B, H, NTB * TS // TC, TC).sum(axis=3, dtype=np.int64)
    excl = np.cumsum(cs, axis=2) - cs                       # [B,H,64]
    phi0q = ((ip[..., 0].astype(np.float64) + np.pi / 2)
             * (MODQ / (2.0 * np.pi)))                      # [B,H]
    vals = ((excl % 65536).astype(np.float64)
            + phi0q[:, :, None]) % MODQ                     # [B,H,64]

    p = np.arange(128)
    tbp = p % 8
    lhsT = np.zeros((128, 64), np.float32)
    for jj in range(8):
        sel = tbp == jj
        lhsT[sel, jj] = -1.0 / 256         # hc (sign undoes the -sin fold)
        for bb in range(4):                # nz partial for batch bb
            lhsT[sel & (p // 32 == bb), 32 + bb * 8 + jj] = 1.0 / 256
    lhsT = lhsT.astype(ml_dtypes.bfloat16)
    m_p = (p % 32) // 8
    cl_p = p // 32
    jj32 = np.arange(32)
    ind_mod = ((cl_p[:, None] == jj32[None, :] // 8) &
               (tbp[:, None] == jj32[None, :] % 8)).astype(np.float32)

    in_maps = []
    for core in range(8):
        b, j = divmod(core, 2)
        hs = slice(j * 64, j * 64 + 64)
        bs = slice(core * 4, core * 4 + 4)  # this core's 4 unique bands
        vb = vals[b, hs]                    # [64 h_local, 64 chunk-ordinal]
        init = np.empty((128, 32), np.float32)
        for g in range(NG):
            init[:, g * 8:(g + 1) * 8] = \
                vb[g * HG:(g + 1) * HG].reshape(128, 8)
        wl = (ind_mod * nbw[b, m_p, 0][:, None]).astype(ml_dtypes.bfloat16)
        ec = nbe[b, m_p, 0].astype(np.float32).reshape(128, 1)
        in_maps.append(dict(q=q16[b, hs], amp=amp8[b, hs],
                            nba=np.ascontiguousarray(
                                nba8[:, bs]).reshape(16, T),
                            nbb=bandsbf[bs], init=init, lhsT=lhsT,
                            wlhsT=wl, ecol=ec))

    res = run_bass_kernel_spmd(nc, in_maps, core_ids=list(range(8)))
    outs = res.results

    sc = np.float32(256.0 / 255.0)
    allo = [r["out"].astype(np.float32) for r in outs]
    # noise[b] = sum of every core's partial for batch b (rows 8+b*8 .. +8)
    noises = []
    for b in range(B):
        nz = allo[0][8 + b * 8:16 + b * 8].reshape(T).copy()
        for core in range(1, 8):
            nz += allo[core][8 + b * 8:16 + b * 8].reshape(T)
        noises.append(nz * sc)
    out = np.empty((B, 1, T), np.float32)
    for b in range(B):
        r0 = outs[2 * b]
        o0, o1 = allo[2 * b], allo[2 * b + 1]
        hc = (o0[0:8].reshape(T) + o1[0:8].reshape(T)) * sc
        noise = noises[b]
        # md[half, j', tl]: j' = cl*8 + tb; t = tb*8192 + (half*4+cl)*1024 + tl
        md = r0["md_out"].astype(np.float32).reshape(2, 4, 8, TC)
        msum = np.ascontiguousarray(md.transpose(2, 0, 1, 3)).reshape(T)
        pgb = pg[b, 0, 0]
        fgb = fg[b, 0, 0]
        tg = (pgb + fgb) * np.float32(0.7)
        out[b, 0] = (hc + msum * noise * pgb + hc * noise * tg
                     + noise * fgb * np.float32(0.3))
    return out
